# revision 87
# baseline (speedup 1.0000x reference)
"""Bass/TRN2 kernel for nn_DeepGeoConvSNN — 8-core data-parallel over batch.

Pipeline per core (16 of 128 batch elements):
  u/v channel-mix as full-rate f32r limb matmuls (PE) -> cross-core sync
  BatchNorm over batch -> LIF layer1 -> conv1d k=15 + 1x1 shortcut (PE,
  f32r hi/lo weight limbs) -> sync BN -> LIF layer2 (8 time-chunks in
  parallel + 28-step warmup; exact because decay 0.5 contracts below f32
  ulp) -> conv1d k=7 + 1x1 (PE) -> sync BN -> LIF layer3 (8 chunks, one
  unified pass, 28-step warmup) -> avgpool32 (one DVE segmented reduce) ->
  sync BN -> folded FC (PE) -> out.

Every LIF scan is decomposed as v_t = L_t - vth*M_t: L is the pure linear
response (one native tensor_tensor_scan per lane, state' = d*state + i) and
M_t = d_t*(M_{t-1} + s_{t-1}) the decayed spike count. The serial per-step
chain is then minimal:
  layer1 (tensor decay): s = (J < C') with C' = (L/vth - 1)/d precomputed,
    Y = J*d issued in s's sem-latency shadow, J' = Y + s  (two sem hops).
  layers2/3 (decay 0.5):  s = (J < C) with C = 2*(L/vth - 1),
    J' = 0.5*J + s as one scalar_tensor_tensor (two sem hops), run as two
    independent half-lane chains interleaved so the DVE never idles.
Layer-2/3 linear responses are never materialized as injections: by
linearity L = al*scan(conv) + beta*g + scan(shortcut) with g_t = 2 - 2^-t,
so the scans run inside the conv1/conv2 windows (off the serial path) and
the BN affine lands post-allreduce as two wide fused ops per half.

decay1 = exp(-1/(35*exp(-(0.8*curv+0.4*tang)))) is computed on host in
float32-faithful arithmetic: the ACT table Exp has ~1.1e-5 relative error,
which measurably flips spikes in this chaotic net (0.14 absmax output error
vs the 0.004 fp32-reordering envelope). All BatchNorm rstd values are
computed on device as reciprocal(sqrt(var+eps)) with one Newton refinement
(ACT Sqrt alone is ~7e-6, same problem; DVE reciprocal is ~6e-8). All of
the above reproduces the pre-rewrite kernel bit-for-bit (same spike flip
counts vs the numpy model, same 1.298e-3 relative error).
"""
import sys
sys.path.insert(0, '/opt/trn_rl_repo')
import numpy as np

import concourse.bass as bass
from concourse import mybir, tile
from concourse.bass_utils import run_bass_kernel_spmd

F32 = mybir.dt.float32
F32R = mybir.dt.float32r
AL = mybir.AluOpType
AF = mybir.ActivationFunctionType
AX = mybir.AxisListType

NCORES = 8
B, C, T = 128, 64, 480
Bs = B // NCORES            # 16 batch per core
C1, C2 = 128, 256
VTH1, VTH2 = 0.15, 0.3
L2_NC, L2_INT, L2_WU = 8, 60, 28
L2_S = 1 + L2_WU + L2_INT   # 93 state slots (slot 0 = zero init)
L2_I = L2_WU + L2_INT       # 92 injection slots
# layer-3 unified chunk grid: 8 chunks x 60 interval, 28-step warmup
# (0.5^28 < f32 ulp of the J state), all 32 (h,b) lanes in one pass.
L3C, L3I, L3W = 8, 60, 28
L3S = L3W + L3I             # 88 serial steps
L3P = 32 + 480              # per-lane C3 columns: 32-col pad | t=0..479
GAM2 = 2.0 / VTH2           # C = GAM2*L - 2 threshold scale (layers 2 & 3)

MAXW_SYNC = 1  # walrus build here rejects >1 sync wait per instruction
NO_CC = False  # profiling mode: replace collectives with local copies


def _split_waits(nc):
    n = 0
    for fn in nc.m.functions:
        for bb in fn.blocks:
            insts = bb.instructions
            out = []
            changed = False
            for inst in insts:
                si = inst.sync_info
                if si is not None and len(si.on_wait) > MAXW_SYNC:
                    w = list(si.on_wait)
                    excess, keep = w[:-MAXW_SYNC], w[-MAXW_SYNC:]
                    for k, sw in enumerate(excess):
                        out.append(mybir.InstNoOp(
                            name=f"{inst.name}-wsplit{k}", engine=inst.engine,
                            sync_info=mybir.SyncInfo(on_wait=[sw], on_update=[]),
                            bass_nofuse=True))
                        n += 1
                    si.on_wait = keep
                    changed = True
                out.append(inst)
            if changed:
                bb.instructions = out
    return n


PHASES = []


def _mark(nc, name):
    PHASES.append((name, len(nc.inst_map)))


def _rsqrt_refined(nc, pool, xe, pdim, fdim, tag):
    """rstd = 1/sqrt(xe) with one Newton step. xe: AP holding var+eps."""
    s0 = pool.tile([pdim, fdim], F32, tag=f"{tag}_s0")
    nc.scalar.activation(s0[:], xe, AF.Sqrt)
    r0 = pool.tile([pdim, fdim], F32, tag=f"{tag}_r0")
    nc.vector.reciprocal(r0[:], s0[:])
    t1 = pool.tile([pdim, fdim], F32, tag=f"{tag}_t1")
    nc.vector.tensor_tensor(t1[:], r0[:], r0[:], AL.mult)
    nc.vector.tensor_tensor(t1[:], t1[:], xe, AL.mult)
    nc.vector.tensor_scalar(t1[:], t1[:], -0.5, 1.5, AL.mult, AL.add)
    r1 = pool.tile([pdim, fdim], F32, tag=f"{tag}_r1")
    nc.vector.tensor_tensor(r1[:], r0[:], t1[:], AL.mult)
    return r1


def build(debug=False, repeat=1, pad_nops=0, race=True):
    nc = bass.Bass(num_devices=NCORES, detect_race_conditions=race)
    ext = {}
    ext["uh_in"] = nc.declare_dram_parameter("uh", [C, Bs * T], F32R, isOutput=False)
    ext["ul_in"] = nc.declare_dram_parameter("ul", [C, Bs * T], F32R, isOutput=False)
    ext["vh_in"] = nc.declare_dram_parameter("vh", [C, Bs * T], F32R, isOutput=False)
    ext["vl_in"] = nc.declare_dram_parameter("vl", [C, Bs * T], F32R, isOutput=False)
    ext["dec_in"] = nc.declare_dram_parameter("dec", [C, Bs * T], F32, isOutput=False)
    ext["wu_w"] = nc.declare_dram_parameter("wu_w", [C, 2 * C], F32R, isOutput=False)
    ext["wv_w"] = nc.declare_dram_parameter("wv_w", [C, 2 * C], F32R, isOutput=False)
    ext["ginj_in"] = nc.declare_dram_parameter("ginj", [C, 1], F32, isOutput=False)
    ext["binj_in"] = nc.declare_dram_parameter("binj", [C, 1], F32, isOutput=False)
    ext["w1p_in"] = nc.declare_dram_parameter("w1p", [C, 30 * C1], F32R, isOutput=False)
    ext["sc1_in"] = nc.declare_dram_parameter("sc1w", [C, 2 * C1], F32R, isOutput=False)
    ext["g1_in"] = nc.declare_dram_parameter("g1", [C1, 1], F32, isOutput=False)
    ext["b1s_in"] = nc.declare_dram_parameter("b1s", [C1, 1], F32, isOutput=False)
    ext["w2_in"] = nc.declare_dram_parameter("w2", [C1, 28 * C1], F32R, isOutput=False)
    ext["sc2_in"] = nc.declare_dram_parameter("sc2w", [C1, 4 * C1], F32R, isOutput=False)
    ext["g2_in"] = nc.declare_dram_parameter("g2", [C1, 2], F32, isOutput=False)
    ext["b2s_in"] = nc.declare_dram_parameter("b2s", [C1, 2], F32, isOutput=False)
    ext["gfc_in"] = nc.declare_dram_parameter("gfc", [C1, 30 * 4], F32, isOutput=False)
    ext["hfc_in"] = nc.declare_dram_parameter("hfc", [4, 1], F32, isOutput=False)
    ext["gv_in"] = nc.declare_dram_parameter("gv", [C1, T], F32, isOutput=False)
    ext["o_out"] = nc.declare_dram_parameter("o", [4, Bs], F32, isOutput=True)
    if debug:
        ext["dbg"] = {
            "dbg_pre": nc.declare_dram_parameter("dbg_pre", [C, Bs * T], F32, isOutput=True),
            "dbg_iinj": nc.declare_dram_parameter("dbg_iinj", [C, Bs * T], F32, isOutput=True),
            "dbg_sp1": nc.declare_dram_parameter("dbg_sp1", [C, Bs * T], F32, isOutput=True),
            "dbg_sp2": nc.declare_dram_parameter("dbg_sp2", [C1, Bs * T], F32, isOutput=True),
            "dbg_sp3": nc.declare_dram_parameter("dbg_sp3", [C1, 2 * Bs * T], F32, isOutput=True),
            "dbg_x": nc.declare_dram_parameter("dbg_x", [C1, 30 * 17], F32, isOutput=True),
        }

    with tile.TileContext(nc, pool_alloc_mode="queue") as tc:
        for rep in range(repeat):
            _emit_body(nc, tc, ext, debug and rep == 0)
        for _ in range(pad_nops):
            nc.vector.nop(hint="pad", nofuse=True)

    _split_waits(nc)
    return nc


def _emit_body(nc, tc, ext, debug):
    dbg = ext.get("dbg") if debug else None
    core_ids = list(range(NCORES))
    o_out = ext["o_out"]

    P = lambda name, side: tc.alloc_tile_pool(name=name, bufs=1, side=side)

    p0 = P("p0", "left")
    pdram = tc.alloc_tile_pool(name="pdram", bufs=1, space="DRAM")

    wwu = p0.tile([C, 2 * C], F32R, tag="wwu")
    wwv = p0.tile([C, 2 * C], F32R, tag="wwv")
    ginj = p0.tile([C, 1], F32, tag="ginj")
    binj = p0.tile([C, 1], F32, tag="binj")
    p_w1 = P("p_w1", "left")
    w1p = p_w1.tile([C, 30 * C1], F32R, tag="w1p")
    sc1w = p_w1.tile([C, 2 * C1], F32R, tag="sc1w")
    nc.sync.dma_start(w1p[:], ext["w1p_in"][:])
    nc.sync.dma_start(sc1w[:], ext["sc1_in"][:])
    g1 = p0.tile([C1, 1], F32, tag="g1")
    b1s = p0.tile([C1, 1], F32, tag="b1s")
    w2 = p0.tile([C1, 28 * C1], F32R, tag="w2")
    sc2w = p0.tile([C1, 4 * C1], F32R, tag="sc2w")
    g2 = p0.tile([C1, 2], F32, tag="g2")
    b2s = p0.tile([C1, 2], F32, tag="b2s")
    gfc = p0.tile([C1, 30 * 4], F32, tag="gfc")
    hfc = p0.tile([4, 1], F32, tag="hfc")
    gv = p0.tile([C1, T], F32, tag="gv")
    half05 = p0.tile([C1, 1], F32, tag="half05")
    nc.gpsimd.memset(half05[:], 0.5)
    for t_, s_ in [(wwu, ext["wu_w"]), (wwv, ext["wv_w"]),
                   (ginj, ext["ginj_in"]), (binj, ext["binj_in"]),
                   (g1, ext["g1_in"]),
                   (b1s, ext["b1s_in"]), (w2, ext["w2_in"]), (sc2w, ext["sc2_in"]),
                   (g2, ext["g2_in"]), (b2s, ext["b2s_in"]), (gfc, ext["gfc_in"]),
                   (hfc, ext["hfc_in"]), (gv, ext["gv_in"])]:
        nc.sync.dma_start(t_[:], s_[:])

    p_dec = P("p_dec", "left")
    dec_sb = p_dec.tile([C, Bs * T], F32, tag="dec_sb")
    stats = p0.tile([C1, 16], F32, tag="stats")      # BN1
    stats2_0 = p0.tile([C1, 16], F32, tag="stats2_0")
    stats2_1 = p0.tile([C1, 16], F32, tag="stats2_1")
    stats2 = [stats2_0, stats2_1]  # BN2 halves

    _mark(nc, "A_premm")
    # ============ phase A ============
    # pre = Wu*(A u) + Wv*(A v) as full-rate f32r limb products (hi/lo limbs
    # of both weights and activations, lo*lo dropped below f32 noise), quarter
    # double-buffered so DMA overlaps the matmuls.
    p_pre = P("p_pre", "right")
    pre_sb = p_pre.tile([C, Bs * T], F32, tag="pre_sb")
    stats1 = p_pre.tile([C, 2 * T], F32, tag="stats1")
    p_uv = P("p_uv", "right")
    QT = Bs * T // 4

    psA = tc.alloc_tile_pool(name="psA", bufs=1, space="PSUM")
    scrA = P("scrA", "right")
    stats1b = scrA.tile([C, 2 * T], F32, tag="stats1b")
    for q in range(4):
        qs = slice(q * QT, (q + 1) * QT)
        uhq = p_uv.tile([C, QT], F32R, tag="uhq", bufs=2, name=f"uhq{q}")
        ulq = p_uv.tile([C, QT], F32R, tag="ulq", bufs=2, name=f"ulq{q}")
        vhq = p_uv.tile([C, QT], F32R, tag="vhq", bufs=2, name=f"vhq{q}")
        vlq = p_uv.tile([C, QT], F32R, tag="vlq", bufs=2, name=f"vlq{q}")
        for t_, s_ in [(uhq, ext["uh_in"]), (ulq, ext["ul_in"]),
                       (vhq, ext["vh_in"]), (vlq, ext["vl_in"])]:
            nc.sync.dma_start(t_[:], s_[:, qs])
        for bq in range(4):
            b = q * 4 + bq
            sl = slice(b * T, (b + 1) * T)
            bsl = slice(bq * T, (bq + 1) * T)
            pre_ps = psA.tile([C, T], F32, tag="pre_ps", bufs=3)
            nc.tensor.matmul(pre_ps[:], wwu[:, 0:C], uhq[:, bsl], start=True, stop=False)
            nc.tensor.matmul(pre_ps[:], wwu[:, C:2 * C], uhq[:, bsl], start=False, stop=False)
            nc.tensor.matmul(pre_ps[:], wwu[:, 0:C], ulq[:, bsl], start=False, stop=False)
            nc.tensor.matmul(pre_ps[:], wwv[:, 0:C], vhq[:, bsl], start=False, stop=False)
            nc.tensor.matmul(pre_ps[:], wwv[:, C:2 * C], vhq[:, bsl], start=False, stop=False)
            nc.tensor.matmul(pre_ps[:], wwv[:, 0:C], vlq[:, bsl], start=False, stop=True)
            nc.scalar.activation(pre_sb[:, sl], pre_ps[:], AF.Copy)
            acc = stats1 if q < 2 else stats1b
            if b in (0, 8):
                nc.scalar.activation(acc[:, T:2 * T], pre_ps[:], AF.Square)
                nc.vector.tensor_scalar(acc[:, 0:T], pre_sb[:, sl], 1.0, None, AL.mult)
            else:
                sq = scrA.tile([C, T], F32, tag="sq_sb", bufs=2)
                nc.scalar.activation(sq[:], pre_ps[:], AF.Square)
                nc.vector.tensor_tensor(acc[:, T:2 * T], acc[:, T:2 * T], sq[:], AL.add)
                nc.vector.tensor_tensor(acc[:, 0:T], acc[:, 0:T], pre_sb[:, sl], AL.add)
    nc.vector.tensor_tensor(stats1[:], stats1[:], stats1b[:], AL.add)
    # dec loads queue behind the premm-critical u/v quarters; needed ~40us later
    for q in range(4):
        nc.sync.dma_start(dec_sb[:, q * (Bs * T // 4):(q + 1) * (Bs * T // 4)],
                          ext["dec_in"][:, q * (Bs * T // 4):(q + 1) * (Bs * T // 4)])
    scrA.release()
    psA.release()
    p_uv.release()

    _mark(nc, "AR1")
    # ---- allreduce 1: per-(c,t) sum & sumsq over batch ----
    ar1_i = pdram.tile([C, 2 * T], F32, tag="ar1_i")
    ar1_o = pdram.tile([C, 2 * T], F32, tag="ar1_o", addr_space="Shared")
    nc.sync.dma_start(ar1_i[:], stats1[:])
    if NO_CC:
        nc.sync.dma_start(ar1_o[:], ar1_i[:])
    else:
        nc.gpsimd.collective_compute("AllReduce", AL.add, replica_groups=[core_ids],
        ins=[ar1_i.opt()], outs=[ar1_o.opt()])
    nc.sync.dma_start(stats1[:], ar1_o[:])

    # conv1's scan targets are allocated here, while the premm u/v space is
    # free, so they never overlap loop-lifetime buffers (which would WAR-block
    # the mid-loop conv1 pipeline).
    L2P = L2_WU + T
    p_c2b = P("p_c2b", "left")
    C2 = p_c2b.tile([C1, Bs * L2P], F32, tag="C2")
    c2bv = C2[:].rearrange("p (b q) -> p b q", b=Bs)
    nc.gpsimd.memset(c2bv[:, :, 0:L2_WU], -2.0)
    c1sum = p_c2b.tile([C1, 2 * Bs], F32, tag="c1sum")
    c1sq = p_c2b.tile([C1, 2 * Bs], F32, tag="c1sq")
    p_lc1 = P("p_lc1", "left")
    Lc1 = p_lc1.tile([C1, Bs * T], F32, tag="Lc1")
    t21 = p_lc1.tile([C1, T], F32, tag="t21")
    lc1v = Lc1[:].rearrange("p (b t) -> p b t", b=Bs)

    _mark(nc, "BNinj_apply")
    # ---- i_inj = pre*R - Q ----
    p_l1 = P("p_l1", "left")
    I1 = p_l1.tile([C, Bs * T], F32, tag="I1")
    N1 = p_l1.tile([C, Bs * 2], F32, tag="N1")     # rotating 2-slot state

    # minimal-scratch BN stats: means in-place in stats1, var in stats1b,
    # rsqrt+Newton in two ping-pong tiles.
    scrB = P("scrB", "right")
    m_t = stats1[:, 0:T]
    nc.vector.tensor_scalar(m_t, m_t, 1.0 / 128.0, None, AL.mult)
    esq = stats1[:, T:2 * T]
    nc.vector.tensor_scalar(esq, esq, 1.0 / 128.0, None, AL.mult)
    # var in-place over esq's slot; Q in-place over the mean's slot
    xe = stats1[:, T:2 * T]
    tmp2 = scrB.tile([C, T], F32, tag="tmp2")
    nc.vector.tensor_tensor(tmp2[:], m_t, m_t, AL.mult)
    nc.vector.tensor_tensor(xe, esq, tmp2[:], AL.subtract)
    nc.vector.tensor_scalar(xe, xe, 1e-5, None, AL.add)
    tbR = scrB.tile([C, T], F32, tag="tbR")
    nc.scalar.activation(tmp2[:], xe, AF.Sqrt)
    nc.vector.reciprocal(tbR[:], tmp2[:])
    nc.vector.tensor_tensor(tmp2[:], tbR[:], tbR[:], AL.mult)
    nc.vector.tensor_tensor(tmp2[:], tmp2[:], xe, AL.mult)
    nc.vector.tensor_scalar(tmp2[:], tmp2[:], -0.5, 1.5, AL.mult, AL.add)
    nc.vector.tensor_tensor(tmp2[:], tbR[:], tmp2[:], AL.mult)   # rstd
    # R/Q carry an extra 1/vth so the linear-response scan directly yields
    # L/vth; the threshold build is then one fused (L' - 1)*rd op per lane.
    R_t = tbR  # rstd's recip scratch becomes R
    nc.vector.tensor_scalar(R_t[:], tmp2[:], ginj[:], 1.0 / VTH1, AL.mult, AL.mult)
    binjs = scrB.tile([C, 1], F32, tag="binjs")
    nc.vector.tensor_scalar(binjs[:], binj[:], 1.0 / VTH1, None, AL.mult)
    Q_t = stats1[:, 0:T]
    nc.vector.tensor_tensor(Q_t, m_t, R_t[:], AL.mult)
    nc.vector.tensor_scalar(Q_t, Q_t, binjs[:], None, AL.subtract)
    i1v3 = I1[:].rearrange("p (b s) -> p b s", b=Bs)
    pre3 = pre_sb[:].rearrange("p (b s) -> p b s", b=Bs)
    Rb = R_t[:].rearrange("p (a s) -> p a s", a=1).to_broadcast([C, Bs, T])
    Qb = Q_t.rearrange("p (a s) -> p a s", a=1).to_broadcast([C, Bs, T])
    nc.vector.tensor_tensor(i1v3, pre3, Rb, AL.mult)
    nc.vector.tensor_tensor(i1v3, i1v3, Qb, AL.subtract)
    if debug:
        nc.sync.dma_start(dbg["dbg_pre"][:], pre_sb[:])
        nc.sync.dma_start(dbg["dbg_iinj"][:], I1[:])
    scrB.release()

    _mark(nc, "L1_scan")
    # ============ phase D: layer-1 LIF scan ============
    # L/M decomposition: v_t = L_t - VTH1*M_t with L the pure linear response
    # (one tensor_tensor_scan per batch element) and M_t = d_t*(M_{t-1}+s_{t-1})
    # the decayed spike count. s_t = (v_t > vth) <=> (M_t < L_t/vth - 1).
    # Serial chain is 3 DVE hops/step (M = J*d, s = M < C, J' = M + s) vs the
    # 4-hop direct form.
    SPW = 496  # 7 zero | 480 spikes | 9 zero
    n1v = N1[:].rearrange("p (b s) -> p b s", b=Bs)
    dv = dec_sb[:].rearrange("p (b s) -> p b s", b=Bs)

    # in-place: I1 -> L (linear response) -> C' = (L/vth - 1)/d.
    # rd = 1/d lets the J-loop compare s=(J < C') straight off the state, so
    # the per-step chain is 2 sem hops (J->s->J') with the J*d multiply
    # hidden under s's semaphore latency. rd reuses pre_sb's storage (pre is
    # dead once I1 is built).
    rd = pre_sb
    nc.vector.reciprocal(rd[:], dec_sb[:])
    for b in range(Bs):
        sl = slice(b * T, (b + 1) * T)
        nc.vector.tensor_tensor_scan(I1[:, sl], dec_sb[:, sl], I1[:, sl],
                                     0.0, AL.mult, AL.add)
        nc.vector.scalar_tensor_tensor(I1[:, sl], I1[:, sl], 1.0, rd[:, sl],
                                       AL.subtract, AL.mult)
    cv = I1[:].rearrange("p (b s) -> p b s", b=Bs)
    p_pre.release()
    p_sp1 = P("p_sp1", "right")
    sppad = p_sp1.tile([C, Bs * SPW], F32R, tag="sppad")
    nc.gpsimd.memset(sppad[:].bitcast(F32), 0.0)
    spav = sppad[:].rearrange("p (s b) -> p s b", b=Bs)  # time-major
    nc.gpsimd.memset(n1v[:, :, 0], 0.0)

    scrD = P("scrD", "right")
    for t in range(T):
        cur = t % 2          # slot holding J[t-1]
        nxt = (t + 1) % 2
        sp_col = spav[:, 7 + t, :]
        nc.vector.tensor_tensor(sp_col, n1v[:, :, cur], cv[:, :, t], AL.is_lt)
        if t < T - 1:
            yt = scrD.tile([C, Bs], F32, tag="yt", bufs=2)
            nc.vector.tensor_tensor(yt[:], n1v[:, :, cur], dv[:, :, t], AL.mult)
            nc.vector.tensor_tensor(n1v[:, :, nxt], yt[:], sp_col, AL.add)
    scrD.release()

    if debug:
        # t-major dump [C, T, Bs]; test.py transposes
        nc.sync.dma_start(dbg["dbg_sp1"][:], sppad[:, 7 * Bs:(7 + T) * Bs].bitcast(F32))
    p_l1.release()

    _mark(nc, "conv1")
    # ============ phase E: conv1 + shortcut + BN1 stats + L2 response ======
    # Like layer 3, the layer-2 injection i2 = al1*c1 + beta + s1 is never
    # materialized: scan(c1) and (2/vth)*scan(s1) are built per (q,b) chunk
    # during conv1 (scans chained across the two q tiles), and C2 is
    # assembled after the BN1 allreduce.

    psE = tc.alloc_tile_pool(name="psE", bufs=1, space="PSUM")
    scrE = P("scrE", "right")
    NT1 = 256  # tile width (fp32r needs moving dim >= 256)
    for q in range(2):
        q0 = 0 if q == 0 else T - NT1            # output cols [0,256) / [224,480)
        cpy = slice(0, NT1) if q == 0 else slice(NT1 - (T - NT1), NT1)
        dst = slice(0, NT1) if q == 0 else slice(NT1, T)
        # all c1 matmuls first: their copies land in the early-placed Lc1, so
        # the whole c1 pipeline overlaps the L1 loop; the s1 block (whose C2
        # target overlays loop-lifetime memory) runs after the loop anyway.
        for b in range(Bs):
            c1_ps = psE.tile([C1, NT1], F32, tag="c1_ps", bufs=6)
            for jl in range(30):
                j = jl // 2
                nc.tensor.matmul(c1_ps[:], w1p[:, jl * C1:(jl + 1) * C1],
                                 spav[:, q0 + j:q0 + j + NT1, b],
                                 start=(jl == 0), stop=(jl == 29))
            nc.scalar.activation(lc1v[:, b, dst], c1_ps[:, cpy], AF.Copy,
                                 accum_out=c1sum[:, q * Bs + b:q * Bs + b + 1])
            sqe = scrE.tile([C1, NT1], F32, tag="sqe", bufs=1)
            nc.scalar.activation(sqe[:, cpy], c1_ps[:, cpy], AF.Square,
                                 accum_out=c1sq[:, q * Bs + b:q * Bs + b + 1])
            ini_l = 0.0 if q == 0 else lc1v[:, b, NT1 - 1:NT1]
            nc.vector.tensor_tensor_scan(lc1v[:, b, dst], half05[:].to_broadcast([C1, dst.stop - dst.start]),
                                         lc1v[:, b, dst], ini_l, AL.mult, AL.add)
        for b in range(Bs):
            s1_ps = psE.tile([C1, NT1], F32, tag="s1_ps", bufs=2)
            for l in range(2):
                nc.tensor.matmul(s1_ps[:], sc1w[:, l * C1:(l + 1) * C1],
                                 spav[:, q0 + 7:q0 + 7 + NT1, b], start=(l == 0), stop=(l == 1))
            nc.scalar.activation(c2bv[:, b, L2_WU + dst.start:L2_WU + dst.stop],
                                 s1_ps[:, cpy], AF.Copy, scale=GAM2)
            ini_s = 0.0 if q == 0 else c2bv[:, b, L2_WU + NT1 - 1:L2_WU + NT1]
            nc.vector.tensor_tensor_scan(c2bv[:, b, L2_WU + dst.start:L2_WU + dst.stop],
                                         half05[:].to_broadcast([C1, dst.stop - dst.start]),
                                         c2bv[:, b, L2_WU + dst.start:L2_WU + dst.stop],
                                         ini_s, AL.mult, AL.add)
    scrE.release()
    psE.release()
    p_sp1.release()

    _mark(nc, "AR2_BN1")
    # ---- allreduce 2: BN1 ----
    nc.vector.tensor_reduce(stats[:, 0:1], c1sum[:], axis=AX.X, op=AL.add)
    nc.vector.tensor_reduce(stats[:, 1:2], c1sq[:], axis=AX.X, op=AL.add)
    ar2_i = pdram.tile([C1, 2], F32, tag="ar2_i")
    ar2_o = pdram.tile([C1, 2], F32, tag="ar2_o", addr_space="Shared")
    nc.sync.dma_start(ar2_i[:], stats[:, 0:2])
    if NO_CC:
        nc.sync.dma_start(ar2_o[:], ar2_i[:])
    else:
        nc.gpsimd.collective_compute("AllReduce", AL.add, replica_groups=[core_ids],
        ins=[ar2_i.opt()], outs=[ar2_o.opt()])
    nc.sync.dma_start(stats[:, 2:4], ar2_o[:])

    NBT = float(B * T)
    nc.vector.tensor_scalar(stats[:, 4:5], stats[:, 2:3], 1.0 / NBT, None, AL.mult)
    nc.vector.tensor_scalar(stats[:, 5:6], stats[:, 3:4], 1.0 / NBT, None, AL.mult)
    nc.vector.tensor_tensor(stats[:, 6:7], stats[:, 4:5], stats[:, 4:5], AL.mult)
    nc.vector.tensor_tensor(stats[:, 6:7], stats[:, 5:6], stats[:, 6:7], AL.subtract)
    nc.vector.tensor_scalar(stats[:, 6:7], stats[:, 6:7], 1e-5, None, AL.add)
    scrF = P("scrF", "left")
    rstd1 = _rsqrt_refined(nc, scrF, stats[:, 6:7], C1, 1, "bn1")
    nc.vector.tensor_scalar(stats[:, 7:8], rstd1[:], g1[:], None, AL.mult)
    nc.vector.tensor_scalar(stats[:, 8:9], stats[:, 7:8], -1.0, None, AL.mult)
    nc.vector.scalar_tensor_tensor(stats[:, 9:10], stats[:, 8:9], stats[:, 4:5],
                                   b1s[:], AL.mult, AL.add)
    nc.vector.tensor_scalar(stats[:, 10:11], stats[:, 7:8], GAM2, None, AL.mult)
    nc.vector.tensor_scalar(stats[:, 11:12], stats[:, 9:10], GAM2, None, AL.mult)
    nc.vector.tensor_scalar(t21[:], gv[:], stats[:, 11:12], -2.0, AL.mult, AL.add)
    scrF.release()

    _mark(nc, "L2_scan")
    # ---- assemble C2 = (2/vth)*(al1*Lc1 + beta*g + Ls1) - 2, then J-loop ----
    c2r = c2bv[:, :, L2_WU:L2P]
    nc.vector.scalar_tensor_tensor(c2r, lc1v[:, :, :], stats[:, 10:11], c2r,
                                   AL.mult, AL.add)
    t21b = t21[:].rearrange("p (a t) -> p a t", a=1).to_broadcast([C1, Bs, T])
    nc.vector.tensor_tensor(c2r, c2r, t21b, AL.add)
    p_lc1.release()

    p_sp2 = P("p_sp2", "right")
    SP2W = 486
    sp2a = p_sp2.tile([C1, Bs * SP2W], F32R, tag="sp2a")
    nc.gpsimd.memset(sp2a[:].bitcast(F32), 0.0)
    sp2v = sp2a[:].rearrange("p (b s) -> p b s", b=Bs)

    p_l2 = P("p_l2", "left")
    N2 = p_l2.tile([C1, 2 * Bs * L2_NC], F32, tag="N2")
    n2v = N2[:].rearrange("p (s b c) -> p s b c", s=2, b=Bs)
    nc.gpsimd.memset(n2v[:, 0, :, :], 0.0)

    scrH = P("scrH", "right")
    SPAN2 = (L2_NC - 1) * L2_INT + 1
    L2S = L2_WU + L2_INT   # 92 serial steps
    # two independent b-half chains interleaved (sA,sB,J'A,J'B): each op's
    # producer is 2+ ops earlier, so the in-order engine never head-of-line
    # stalls and the DVE stays saturated.
    HB = Bs // 2
    for s in range(L2S):
        cur, nxt = s % 2, (s + 1) % 2
        sp_locs = []
        for g in range(2):
            bsl = slice(g * HB, (g + 1) * HB)
            if s >= L2_WU:
                sp_loc = sp2v[:, bsl, s - (L2_WU - 3):s - (L2_WU - 3) + SPAN2:L2_INT]
            else:
                spscr = scrH.tile([C1, HB * L2_NC], F32, tag=f"spscr{g}",
                                  name=f"spscr{g}", bufs=2)
                sp_loc = spscr[:].rearrange("p (b c) -> p b c", b=HB)
            cs = c2bv[:, bsl, s:s + SPAN2:L2_INT]
            nc.vector.tensor_tensor(sp_loc, n2v[:, cur, bsl, :], cs, AL.is_lt)
            sp_locs.append(sp_loc)
        if s < L2S - 1:
            for g in range(2):
                bsl = slice(g * HB, (g + 1) * HB)
                nc.vector.scalar_tensor_tensor(n2v[:, nxt, bsl, :], n2v[:, cur, bsl, :],
                                               0.5, sp_locs[g], AL.mult, AL.add)
    scrH.release()
    p_l2.release()
    p_c2b.release()
    p_dec.release()
    p_w1.release()
    if debug:
        dsp2 = dbg["dbg_sp2"][:].rearrange("p (b s) -> p b s", b=Bs)
        nc.sync.dma_start(dsp2[:, :, :], sp2v[:, :, 3:3 + T].bitcast(F32))

    _mark(nc, "conv2_BN2")
    # ============ phase H: conv2 + BN2 stats + L3 linear response ============
    # The layer-3 injection i3 = al2*c2 + beta + s2 is never materialized.
    # By linearity of the 0.5-decay scan, its linear response is
    #   L3 = al2*scan(c2) + beta*g + scan(s2),   g_t = 2 - 2^-t,
    # so scan(c2) and scale*scan(s2) are computed per (h,b) during conv2 (off
    # the critical path), and the J-loop threshold C3 = (2/VTH2)*L3 - 2 is
    # assembled with two wide ops per half once the BN2 allreduce lands.
    p_c3 = P("p_c3", "left")
    C3 = p_c3.tile([C1, 2 * Bs * L3P], F32, tag="C3")
    X = p_c3.tile([C1, 30 * 17], F32, tag="X")
    c3v = C3[:].rearrange("p (j q) -> p j q", j=2 * Bs)
    p_lc3 = P("p_lc3", "left")
    Lc3 = p_lc3.tile([C1, 2 * Bs * T], F32, tag="Lc3")
    lc3v = Lc3[:].rearrange("p (j t) -> p j t", j=2 * Bs)
    nc.gpsimd.memset(c3v[:, :, 0:32], -2.0)

    t2h = [p_c3.tile([C1, T], F32, tag=f"t2h{h}", name=f"t2h{h}")
           for h in range(2)]
    psH = tc.alloc_tile_pool(name="psH", bufs=1, space="PSUM")
    scrI = P("scrI", "right")
    c2sum = [p_c3.tile([C1, Bs], F32, tag=f"c2sum{h}", name=f"c2sum{h}")
             for h in range(2)]
    c2sq = [p_c3.tile([C1, Bs], F32, tag=f"c2sq{h}", name=f"c2sq{h}")
            for h in range(2)]
    for h in range(2):
        for b in range(Bs):
            j = h * Bs + b
            c2_ps = psH.tile([C1, T], F32, tag="c2_ps", bufs=3)
            for kl in range(14):
                k = kl // 2
                blk = (k * 2 + h) * 2 + (kl % 2)
                nc.tensor.matmul(c2_ps[:], w2[:, blk * C1:(blk + 1) * C1],
                                 sp2v[:, b, k:k + T],
                                 start=(kl == 0), stop=(kl == 13))
            s2_ps = psH.tile([C1, T], F32, tag="s2_ps", bufs=2)
            for l in range(2):
                nc.tensor.matmul(s2_ps[:], sc2w[:, (h * 2 + l) * C1:(h * 2 + l + 1) * C1],
                                 sp2v[:, b, 3:3 + T], start=(l == 0), stop=(l == 1))
            c2c = scrI.tile([C1, T], F32, tag="c2c", bufs=6)
            nc.scalar.activation(c2c[:], c2_ps[:], AF.Copy, accum_out=c2sum[h][:, b:b + 1])
            sqi = scrI.tile([C1, T], F32, tag="sqi", bufs=1)
            nc.scalar.activation(sqi[:], c2_ps[:], AF.Square, accum_out=c2sq[h][:, b:b + 1])
            nc.vector.tensor_tensor_scan(lc3v[:, j, :], half05[:].to_broadcast([C1, T]),
                                         c2c[:], 0.0, AL.mult, AL.add)
            s2s = scrI.tile([C1, T], F32, tag="s2s", bufs=1)
            nc.vector.tensor_scalar(s2s[:], s2_ps[:], GAM2, None, AL.mult)
            nc.vector.tensor_tensor_scan(c3v[:, j, 32:L3P], half05[:].to_broadcast([C1, T]),
                                         s2s[:], 0.0, AL.mult, AL.add)
        # issue this half's BN2 allreduce now so it overlaps the other half
        st2 = stats2[h]
        nc.vector.tensor_reduce(st2[:, 0:1], c2sum[h][:], axis=AX.X, op=AL.add)
        nc.vector.tensor_reduce(st2[:, 1:2], c2sq[h][:], axis=AX.X, op=AL.add)
        ar3_i = pdram.tile([C1, 2], F32, tag=f"ar3_i{h}", name=f"ar3_i{h}")
        ar3_o = pdram.tile([C1, 2], F32, tag=f"ar3_o{h}", name=f"ar3_o{h}",
                           addr_space="Shared")
        nc.sync.dma_start(ar3_i[:], st2[:, 0:2])
        if NO_CC:
            nc.sync.dma_start(ar3_o[:], ar3_i[:])
        else:
            nc.gpsimd.collective_compute("AllReduce", AL.add, replica_groups=[core_ids],
            ins=[ar3_i.opt()], outs=[ar3_o.opt()])
        nc.sync.dma_start(st2[:, 2:4], ar3_o[:])
        # BN2 scalars + C3 assembly for this half; for h=0 this overlaps the
        # other half's matmuls (c2c is buffered deep enough that the ACT
        # copies never stall behind these DVE ops)
        o0 = 2
        nc.vector.tensor_scalar(st2[:, o0 + 2:o0 + 3], st2[:, o0:o0 + 1], 1.0 / NBT, None, AL.mult)
        nc.vector.tensor_scalar(st2[:, o0 + 3:o0 + 4], st2[:, o0 + 1:o0 + 2], 1.0 / NBT, None, AL.mult)
        nc.vector.tensor_tensor(st2[:, o0 + 4:o0 + 5], st2[:, o0 + 2:o0 + 3], st2[:, o0 + 2:o0 + 3], AL.mult)
        nc.vector.tensor_tensor(st2[:, o0 + 4:o0 + 5], st2[:, o0 + 3:o0 + 4], st2[:, o0 + 4:o0 + 5], AL.subtract)
        nc.vector.tensor_scalar(st2[:, o0 + 4:o0 + 5], st2[:, o0 + 4:o0 + 5], 1e-5, None, AL.add)
        scrJ = P(f"scrJ{h}", "right")
        rstd2 = _rsqrt_refined(nc, scrJ, st2[:, o0 + 4:o0 + 5], C1, 1, f"bn2{h}")
        al2 = st2[:, o0 + 5:o0 + 6]
        nc.vector.tensor_scalar(al2, rstd2[:], g2[:, h:h + 1], None, AL.mult)
        nc.vector.tensor_scalar(st2[:, o0 + 6:o0 + 7], al2, -1.0, None, AL.mult)
        nc.vector.scalar_tensor_tensor(st2[:, o0 + 7:o0 + 8], st2[:, o0 + 6:o0 + 7],
                                       st2[:, o0 + 2:o0 + 3], b2s[:, h:h + 1], AL.mult, AL.add)
        alg = st2[:, o0 + 8:o0 + 9]     # al2 * 2/vth
        nc.vector.tensor_scalar(alg, al2, GAM2, None, AL.mult)
        beg = st2[:, o0 + 9:o0 + 10]    # beta * 2/vth
        nc.vector.tensor_scalar(beg, st2[:, o0 + 7:o0 + 8], GAM2, None, AL.mult)
        nc.vector.tensor_scalar(t2h[h][:], gv[:], beg, -2.0, AL.mult, AL.add)
        scrJ.release()
        jsl = slice(h * Bs, (h + 1) * Bs)
        c3r = c3v[:, jsl, 32:L3P]
        nc.vector.scalar_tensor_tensor(c3r, lc3v[:, jsl, :], alg, c3r, AL.mult, AL.add)
        t2b = t2h[h][:].rearrange("p (a t) -> p a t", a=1).to_broadcast([C1, Bs, T])
        nc.vector.tensor_tensor(c3r, c3r, t2b, AL.add)

    scrI.release()
    psH.release()
    p_lc3.release()

    _mark(nc, "L3_pool")
    # ============ phase I: unified layer-3 J-loop + pooling ============
    p_sp3 = P("p_sp3", "right")
    sp3n = p_sp3.tile([C1, 2 * Bs * T], F32, tag="sp3n")
    sp3v = sp3n[:].rearrange("p (j t) -> p j t", j=2 * Bs)
    p_n3 = P("p_n3", "right")
    N3 = p_n3.tile([C1, 2 * 2 * Bs * L3C], F32, tag="N3")
    n3v = N3[:].rearrange("p (s j c) -> p s j c", s=2, j=2 * Bs)
    nc.gpsimd.memset(n3v[:, 0, :, :], 0.0)

    scrK = P("scrK", "right")
    SPAN3 = (L3C - 1) * L3I + 1
    for s in range(L3S):
        cur, nxt = s % 2, (s + 1) % 2
        sp_locs = []
        for g in range(2):
            jsl = slice(g * Bs, (g + 1) * Bs)
            if s >= L3W:
                t0 = s - L3W
                sp_loc = sp3v[:, jsl, t0:t0 + SPAN3:L3I]
            else:
                spscr = scrK.tile([C1, Bs * L3C], F32, tag=f"spscr3{g}",
                                  name=f"spscr3{g}", bufs=2)
                sp_loc = spscr[:].rearrange("p (j c) -> p j c", j=Bs)
            cs = c3v[:, jsl, s + 4:s + 4 + SPAN3:L3I]
            nc.vector.tensor_tensor(sp_loc, n3v[:, cur, jsl, :], cs, AL.is_lt)
            sp_locs.append(sp_loc)
        if s < L3S - 1:
            for g in range(2):
                jsl = slice(g * Bs, (g + 1) * Bs)
                nc.vector.scalar_tensor_tensor(n3v[:, nxt, jsl, :], n3v[:, cur, jsl, :],
                                               0.5, sp_locs[g], AL.mult, AL.add)
    scrK.release()
    p_n3.release()
    if debug:
        nc.sync.dma_start(dbg["dbg_sp3"][:], sp3n[:])

    # pooling: mean over 32-wide windows -> X[:, h*15+w, b]
    inv = sp3n[:].rearrange("p (h b w t) -> p h b w t", h=2, b=Bs, w=15)
    xv = X[:].rearrange("p (h w r) -> p h r w", h=2, w=15, r=17)
    nc.vector.tensor_reduce(xv[:, :, 0:Bs, :], inv, axis=AX.X, op=AL.add)
    p_sp3.release()
    p_sp2.release()

    _mark(nc, "fc")
    # ============ phase J: prefc BN + FC ============
    xv3 = X[:].rearrange("p (q r) -> p q r", r=17)
    scrL = P("scrL", "left")
    xsq = scrL.tile([C1, 30 * 16], F32, tag="xsq")
    xsqv = xsq[:].rearrange("p (q r) -> p q r", r=16)
    nc.scalar.activation(xsqv[:, :, :], xv3[:, :, 0:Bs], AF.Square)
    st4 = scrL.tile([C1, 4 * 30], F32, tag="st4")
    nc.vector.tensor_reduce(st4[:, 0:30], xv3[:, :, 0:Bs], axis=AX.X, op=AL.add)
    nc.vector.tensor_reduce(st4[:, 30:60], xsqv[:, :, :], axis=AX.X, op=AL.add)
    ar4_i = pdram.tile([C1, 60], F32, tag="ar4_i")
    ar4_o = pdram.tile([C1, 60], F32, tag="ar4_o", addr_space="Shared")
    nc.sync.dma_start(ar4_i[:], st4[:, 0:60])
    if NO_CC:
        nc.sync.dma_start(ar4_o[:], ar4_i[:])
    else:
        nc.gpsimd.collective_compute("AllReduce", AL.add, replica_groups=[core_ids],
        ins=[ar4_i.opt()], outs=[ar4_o.opt()])
    nc.sync.dma_start(st4[:, 60:120], ar4_o[:])
    m4 = scrL.tile([C1, 30], F32, tag="m4")
    nc.vector.tensor_scalar(m4[:], st4[:, 60:90], 1.0 / 128.0, None, AL.mult)
    e4 = scrL.tile([C1, 30], F32, tag="e4")
    nc.vector.tensor_scalar(e4[:], st4[:, 90:120], 1.0 / 128.0, None, AL.mult)
    v4 = scrL.tile([C1, 30], F32, tag="v4")
    nc.vector.tensor_tensor(v4[:], m4[:], m4[:], AL.mult)
    nc.vector.tensor_tensor(v4[:], e4[:], v4[:], AL.subtract)
    nc.vector.tensor_scalar(v4[:], v4[:], 1.0 / 1024.0, 1e-5, AL.mult, AL.add)
    rstd4 = _rsqrt_refined(nc, scrL, v4[:], C1, 30, "bnfc")
    G = scrL.tile([C1, 30 * 4], F32, tag="G")
    gv_ = G[:].rearrange("p (q r) -> p q r", r=4)
    gfcv = gfc[:].rearrange("p (q r) -> p q r", r=4)
    rsb = rstd4[:].rearrange("p (q a) -> p q a", a=1).to_broadcast([C1, 30, 4])
    nc.vector.tensor_tensor(gv_, gfcv, rsb, AL.mult)
    nc.vector.tensor_scalar(xv3[:, :, 16], m4[:], 1.0, None, AL.mult)
    if debug:
        nc.sync.dma_start(dbg["dbg_x"][:], X[:])

    psJ = tc.alloc_tile_pool(name="psJ", bufs=1, space="PSUM")
    fc_ps = psJ.tile([4, 17], F32, tag="fc_ps")
    for cch in range(30):
        nc.tensor.matmul(fc_ps[:], gv_[:, cch, :], xv3[:, cch, :],
                         start=(cch == 0), stop=(cch == 29))
    mcol = scrL.tile([4, 1], F32, tag="mcol")
    nc.scalar.activation(mcol[:], fc_ps[:, 16:17], AF.Copy)
    ofin = scrL.tile([4, Bs], F32, tag="ofin")
    nc.vector.tensor_scalar(ofin[:], fc_ps[:, 0:16], mcol[:], None, AL.subtract)
    nc.vector.tensor_scalar(ofin[:], ofin[:], hfc[:], None, AL.add)
    nc.sync.dma_start(o_out[:], ofin[:])
    psJ.release()
    scrL.release()
    p_c3.release()

    p0.release()
    pdram.release()


# ======================= host side =======================

def _host_prep(inputs):
    f64 = np.float64
    f32 = np.float32
    feats = np.asarray(inputs['features'])
    A = np.asarray(inputs['A_norm']); Wu = np.asarray(inputs['Wu_w']); Wv = np.asarray(inputs['Wv_w'])
    conv1_w = np.asarray(inputs['conv1_w']); sc1_w = np.asarray(inputs['sc1_w'])
    conv2_w = np.asarray(inputs['conv2_w']); sc2_w = np.asarray(inputs['sc2_w'])

    u = feats[..., 0]; v = feats[..., 1]; curv = feats[..., 2]; tang = feats[..., 3]
    e = np.exp(-(f32(0.8) * curv + f32(0.4) * tang), dtype=f32)
    tau = (f32(35.0) * e).astype(f32)
    dec = np.exp(f32(-1.0) / tau, dtype=f32)

    def _trunc(x, nbits=10):
        xi = np.ascontiguousarray(x, f32).view(np.uint32)
        return (xi & (np.uint32(0xFFFFFFFF) << np.uint32(23 - nbits))).view(f32)

    def _limbs(w):
        hi = _trunc(w)
        lo = _trunc((w - hi).astype(f32))
        return hi, lo

    w1p = np.zeros((C, 30 * C1), f32)
    for j in range(15):
        hi, lo = _limbs(conv1_w[:, :, j].T.astype(f32))
        w1p[:, (2 * j) * C1:(2 * j + 1) * C1] = hi
        w1p[:, (2 * j + 1) * C1:(2 * j + 2) * C1] = lo
    w2 = np.zeros((C1, 28 * C1), f32)
    for k in range(7):
        for h in range(2):
            hi, lo = _limbs(conv2_w[h * C1:(h + 1) * C1, :, k].T.astype(f32))
            blk = (k * 2 + h) * 2
            w2[:, blk * C1:(blk + 1) * C1] = hi
            w2[:, (blk + 1) * C1:(blk + 2) * C1] = lo
    sc2 = np.zeros((C1, 4 * C1), f32)
    for h in range(2):
        hi, lo = _limbs(sc2_w[h * C1:(h + 1) * C1, :, 0].T.astype(f32))
        sc2[:, (h * 2) * C1:(h * 2 + 1) * C1] = hi
        sc2[:, (h * 2 + 1) * C1:(h * 2 + 2) * C1] = lo

    gp = np.asarray(inputs['prefc_g']).astype(f64)
    bp = np.asarray(inputs['prefc_b']).astype(f64)
    fcw = np.asarray(inputs['fc_w']).astype(f64)
    gfc = np.zeros((C1, 30 * 4), f32)
    for half in range(2):
        for w in range(15):
            cch = half * 15 + w
            fidx = (half * C1 + np.arange(C1)) * 15 + w
            gfc[:, cch * 4:(cch + 1) * 4] = (fcw[:, fidx] * gp[fidx] / 32.0).T.astype(f32)
    hfc = (np.asarray(inputs['fc_b']).astype(f64) + fcw @ bp).astype(f32).reshape(4, 1)

    shared = {
        "wu_w": np.concatenate(_limbs(np.ascontiguousarray(
            (A.astype(f64) @ Wu.T.astype(f64)).astype(f32))), axis=1),
        "wv_w": np.concatenate(_limbs(np.ascontiguousarray(
            (A.astype(f64) @ Wv.T.astype(f64)).astype(f32))), axis=1),
        "ginj": np.asarray(inputs['bn_inj_g']).astype(f32).reshape(C, 1),
        "binj": np.asarray(inputs['bn_inj_b']).astype(f32).reshape(C, 1),
        "w1p": w1p,
        "sc1w": np.concatenate(_limbs(np.ascontiguousarray(sc1_w[:, :, 0].T.astype(f32))), axis=1),
        "g1": np.asarray(inputs['bn1_g']).astype(f32).reshape(C1, 1),
        "b1s": (np.asarray(inputs['bn1_b']).astype(f64)
                + np.asarray(inputs['sc1_b']).astype(f64)).astype(f32).reshape(C1, 1),
        "w2": w2,
        "sc2w": sc2,
        "g2": np.ascontiguousarray(np.asarray(inputs['bn2_g']).astype(f32).reshape(2, C1).T),
        "b2s": np.ascontiguousarray(
            (np.asarray(inputs['bn2_b']).astype(f64)
             + np.asarray(inputs['sc2_b']).astype(f64)).astype(f32).reshape(2, C1).T),
        "gfc": gfc,
        "hfc": hfc,
        # g_t = scan(0.5, ones)_t = 2 - 2^-t (exact f32 trajectory)
        "gv": np.broadcast_to((f64(2.0) - np.power(2.0, -np.arange(T, dtype=f64))
                               ).astype(f32), (C1, T)).copy(),
    }
    in_maps = []
    for k in range(NCORES):
        bs = slice(k * Bs, (k + 1) * Bs)
        m = dict(shared)
        uc = np.ascontiguousarray(u[bs].transpose(1, 0, 2).reshape(C, Bs * T))
        vc = np.ascontiguousarray(v[bs].transpose(1, 0, 2).reshape(C, Bs * T))
        m["uh"], m["ul"] = _limbs(uc)
        m["vh"], m["vl"] = _limbs(vc)
        m["dec"] = np.ascontiguousarray(dec[bs].transpose(1, 0, 2).reshape(C, Bs * T))
        in_maps.append(m)
    return in_maps


_NC_CACHE = {}


def _get_nc(debug=False, repeat=1):
    key = (debug, repeat)
    if key not in _NC_CACHE:
        _NC_CACHE[key] = build(debug=debug, repeat=repeat)
    return _NC_CACHE[key]


def run(inputs, debug=False, repeat=1):
    in_maps = _host_prep(inputs)
    nc = _get_nc(debug=debug, repeat=repeat)
    res = run_bass_kernel_spmd(nc, in_maps, list(range(NCORES)))
    out = np.concatenate([res.results[k]["o"].T for k in range(NCORES)], axis=0)
    return out.astype(np.float32), res


def kernel(**inputs) -> np.ndarray:
    out, _ = run(inputs)
    return out

